# revision 20
# baseline (speedup 1.0000x reference)
"""Trainium2 Bass kernel for nn_BondWeight (symmetric edge-weight scatter).

Problem: out[b, src[b,e]+1, dst[b,e]+1] = w[b,e] and
         out[b, dst[b,e]+1, src[b,e]+1] = w[b,e]  (set semantics, XLA-CPU
         last-write-wins order: full scatter-1 pass then scatter-2 pass),
         where w = weights[bond_type], out is [1024, 256, 256] f32 zeros.

Strategy (8 NeuronCores, data-parallel over batch, 128 batches/core):
  Host: gather weights, compute write positions, dedup duplicate positions
        keeping only the final writer (reproduces XLA-CPU set semantics),
        then pack per (batch-pair, partition) scatter lists. f32 values are
        split into lo/hi int16 halves (bit-exact).
  Device (per core): GPSIMD `local_scatter` builds zeroed + scattered
        int16 tiles in Q7-local RAM and streams them to SBUF. Per-instruction
        overhead (~0.8us) dominates, so each instruction covers TWO batches:
        [128 partitions x 2046 int16] = batch k2 (full 1024) + batch k2+1
        (1022 of 1024; the missing f32 per partition - row 2p+1, col 255 -
        is covered by one strided patch DMA). Tiles are DMAed contiguously
        to the output, double-buffered so GPSIMD and DMA overlap.
"""

import numpy as np

B, E, T, N = 1024, 512, 8, 256
M = 8                      # cores
BL = B // M                # 128 batches per core
NPAIR = BL // 2            # 64 batch pairs per core
NN = N * N                 # 65536
PARTS = 128                # partition p holds rows 2p, 2p+1
BELEMS = 2 * N * 2         # 1024 int16 per partition per batch
ELEMS = 2046               # int16 per partition per pair instruction (max)
NBUF = 8                   # tile double-buffering depth

_nc_cache = {}


def _prepare_scatter(weights, bond_src, bond_dst, bond_type):
    """Returns (idx, dat, patch, niw).

    idx/dat: int16 [M, PARTS, NPAIR*niw] scatter slots (idx==-1 padded).
    patch:   f32  [M, PARTS, NPAIR]: value of (batch 2k+1, row 2p+1, col
             255), i.e. the one f32 per partition that doesn't fit in the
             2046-int16 pair tile. Mostly zero.
    """
    w = np.ascontiguousarray(weights, dtype=np.float32)[np.asarray(bond_type)]
    s = np.asarray(bond_src, dtype=np.int64) + 1
    d = np.asarray(bond_dst, dtype=np.int64) + 1
    bb = np.arange(B, dtype=np.int64)[:, None]
    key = np.concatenate([bb * NN + s * N + d, bb * NN + d * N + s],
                         axis=1).ravel()
    order = np.tile(np.arange(2 * E, dtype=np.int64), B)
    vals = np.concatenate([w, w], axis=1).ravel()

    sortidx = np.lexsort((order, key))
    ksort = key[sortidx]
    is_last = np.empty(len(ksort), dtype=bool)
    is_last[:-1] = ksort[1:] != ksort[:-1]
    is_last[-1] = True
    sel = sortidx[is_last]            # final writer of each position
    fkey = key[sel]
    fval = vals[sel]

    gb = fkey // NN                   # global batch
    q = fkey % NN
    r = q // N                        # row
    c = q % N                         # col
    m = gb // BL                      # core
    b = gb % BL                       # batch within core
    pr = b // 2                       # pair index
    h = b % 2                         # half within pair
    p = r // 2                        # partition
    qq = (r % 2) * N + c              # f32 position within partition tile

    # the one position per partition that doesn't fit: h==1 and qq==511
    is_patch = (h == 1) & (qq == 2 * N - 1)

    patch = np.zeros((M, PARTS, NPAIR), dtype=np.float32)
    patch[m[is_patch], p[is_patch], pr[is_patch]] = fval[is_patch]

    mk = ~is_patch
    m2, pr2, p2, h2, qq2, fv2 = m[mk], pr[mk], p[mk], h[mk], qq[mk], fval[mk]
    base = (h2 * 1024 + 2 * qq2).astype(np.int64)   # int16 index in pair tile

    grp = (m2 * NPAIR + pr2) * PARTS + p2
    o2 = np.argsort(grp, kind="stable")
    grp_s = grp[o2]
    n_ent = len(grp_s)
    new_grp = np.empty(n_ent, dtype=bool)
    new_grp[0] = True
    new_grp[1:] = grp_s[1:] != grp_s[:-1]
    gstart = np.maximum.accumulate(np.where(new_grp, np.arange(n_ent), 0))
    cc = np.arange(n_ent) - gstart

    niw = 2 * (int(cc.max()) + 1)
    if niw % 2:
        niw += 1

    bits = fv2[o2].view(np.uint32).astype(np.int64)
    lo = (bits & 0xFFFF).astype(np.uint16).view(np.int16)
    hi = ((bits >> 16) & 0xFFFF).astype(np.uint16).view(np.int16)
    bs = base[o2]
    ms, ps, prs = m2[o2], p2[o2], pr2[o2]

    idx = np.full((M, PARTS, NPAIR * niw), -1, dtype=np.int16)
    dat = np.zeros((M, PARTS, NPAIR * niw), dtype=np.int16)
    col = prs * niw + 2 * cc
    idx[ms, ps, col] = bs.astype(np.int16)
    idx[ms, ps, col + 1] = (bs + 1).astype(np.int16)
    dat[ms, ps, col] = lo
    dat[ms, ps, col + 1] = hi
    return idx, dat, patch, niw


def _build_nc(niw):
    import concourse.bass as bass
    import concourse.mybir as mybir
    from concourse import library_config

    nc = bass.Bass("TRN2", target_bir_lowering=False)
    idx_t = nc.dram_tensor("lsidx", [PARTS, NPAIR * niw], mybir.dt.int16,
                           kind="ExternalInput")
    dat_t = nc.dram_tensor("lsdat", [PARTS, NPAIR * niw], mybir.dt.int16,
                           kind="ExternalInput")
    pat_t = nc.dram_tensor("lspatch", [PARTS, NPAIR], mybir.dt.float32,
                           kind="ExternalInput")
    # int16 view of the [BL, 256, 256] f32 output: batch b, partition p ->
    # int16 elements [b*PARTS*1024 + p*1024, +1024) (f32 rows 2p, 2p+1)
    out_t = nc.dram_tensor("out", [BL * PARTS, BELEMS], mybir.dt.int16,
                           kind="ExternalOutput")
    # f32 view for the patch DMA (same buffer would be ideal; instead use
    # an int16 AP pair per element): element (p, k) of patch goes to f32
    # position (2k+1)*NN + p*512 + 511 == int16 offset ((2k+1)*NN+p*512+511)*2
    with (
        nc.sbuf_tensor("idx_sb", [PARTS, NPAIR * niw], mybir.dt.int16) as idx_sb,
        nc.sbuf_tensor("dat_sb", [PARTS, NPAIR * niw], mybir.dt.int16) as dat_sb,
        nc.sbuf_tensor("pat_sb", [PARTS, NPAIR], mybir.dt.float32) as pat_sb,
        nc.sbuf_tensor("dst_sb", [PARTS, NBUF * ELEMS], mybir.dt.int16) as dst_sb,
        nc.semaphore("pat_sem") as pat_sem,
        nc.semaphore("ls_sem") as ls_sem,
        nc.semaphore("dma_sem") as dma_sem,
        nc.semaphore("ch0") as ch0,
        nc.semaphore("ch1") as ch1,
        nc.semaphore("ch2") as ch2,
        nc.semaphore("ch3") as ch3,
        nc.Block(no_gpsimd_drain=True) as block,
    ):
        # input DMAs arrive in chunks of ICH pairs, each gated by its OWN
        # semaphore (a shared counter would be racy under DMA-completion
        # reordering), so the first local_scatter can start early
        ch_sems = [ch0, ch1, ch2, ch3]
        NCH = len(ch_sems)
        ICH = NPAIR // NCH

        @block.gpsimd
        def _(gpsimd):
            gpsimd.load_library(library_config.local_scatter)
            # dummy call pays the ~6us first-use IRAM load of the library
            # while the input DMAs are still in flight. Reads uninitialized
            # dst_sb (not a concurrent DMA target); all scatter byte-offsets
            # are uint16 so they stay inside the 64KB Q7 scratch; the dst
            # region is fully rewritten by pair 0.
            gpsimd.local_scatter(
                out_ap=dst_sb[:, 0:2], data_ap=dst_sb[:, 4:6],
                idxs_ap=dst_sb[:, 8:10],
                channels=PARTS, num_elems=2, num_idxs=2)
            for k in range(NPAIR):
                if k % ICH == 0:
                    gpsimd.wait_ge(ch_sems[k // ICH], 32)
                if k >= NBUF and k % 2 == 0:
                    # pairs up to k-NBUF+1 have had their tile DMAs (ap1+ap2,
                    # 2 x 16 incs each) complete; covers buffer reuse for
                    # pairs k and k+1
                    gpsimd.wait_ge(dma_sem, 32 * (k - NBUF + 2))
                kb = (k % NBUF) * ELEMS
                gpsimd.local_scatter(
                    out_ap=dst_sb[:, kb:kb + ELEMS],
                    data_ap=dat_sb[:, k * niw:(k + 1) * niw],
                    idxs_ap=idx_sb[:, k * niw:(k + 1) * niw],
                    channels=PARTS,
                    num_elems=ELEMS,
                    num_idxs=niw,
                ).then_inc(ls_sem, 1)

        @block.sync
        def _(sync):
            W = ICH * niw
            sync.dma_start(idx_sb[:, 0:W], idx_t[:, 0:W]).then_inc(ch0, 16)
            sync.dma_start(dat_sb[:, 0:W], dat_t[:, 0:W]).then_inc(ch0, 16)
            sync.dma_start(pat_sb[:], pat_t[:]).then_inc(pat_sem, 16)
            for c in range(1, NCH):
                cs = slice(c * W, (c + 1) * W)
                sync.dma_start(idx_sb[:, cs], idx_t[:, cs]) \
                    .then_inc(ch_sems[c], 16)
                sync.dma_start(dat_sb[:, cs], dat_t[:, cs]) \
                    .then_inc(ch_sems[c], 16)
            sync.wait_ge(pat_sem, 16)
            pat_src = pat_sb[:].bitcast(mybir.dt.int16)  # [128, 2*NPAIR]
            for k in range(NPAIR):
                sync.wait_ge(ls_sem, k + 1)
                kb = (k % NBUF) * ELEMS
                # batch 2k: full 1024 int16 per partition
                ap1 = bass.AP(out_t, (2 * k) * PARTS * BELEMS,
                              [[BELEMS, PARTS], [1, BELEMS]])
                sync.dma_start(ap1, dst_sb[:, kb:kb + 1024]) \
                    .then_inc(dma_sem, 16)
                # batch 2k+1: first 1022 int16 per partition
                ap2 = bass.AP(out_t, (2 * k + 1) * PARTS * BELEMS,
                              [[BELEMS, PARTS], [1, 1022]])
                sync.dma_start(ap2, dst_sb[:, kb + 1024:kb + 2046]) \
                    .then_inc(dma_sem, 16)
                # patch: the missing f32 (row 2p+1, col 255) of batch 2k+1,
                # one small DMA per pair so no big-FIFO stall; counted on
                # pat_sem so tile-buffer reuse waits see only ap1/ap2
                ap3 = bass.AP(out_t,
                              (2 * k + 1) * PARTS * BELEMS + BELEMS - 2,
                              [[BELEMS, PARTS], [1, 2]])
                sync.dma_start(ap3, pat_src[:, 2 * k:2 * k + 2]) \
                    .then_inc(pat_sem, 16)
            sync.wait_ge(dma_sem, 32 * NPAIR)
            sync.wait_ge(pat_sem, 16 + 16 * NPAIR)

    from concourse.library_overlay import lower_extended_insts
    lower_extended_insts(nc)
    return nc


def _get_nc(niw):
    if niw not in _nc_cache:
        _nc_cache[niw] = _build_nc(niw)
    return _nc_cache[niw]


def run_with_stats(inputs, trace=False):
    """Run the kernel; returns (output [B,N,N] f32, exec_time_ns or None)."""
    from concourse.bass_utils import run_bass_kernel_spmd

    idx, dat, patch, niw = _prepare_scatter(
        inputs["weights"], inputs["bond_src"],
        inputs["bond_dst"], inputs["bond_type"])
    nc = _get_nc(niw)
    in_maps = [{"lsidx": np.ascontiguousarray(idx[m]),
                "lsdat": np.ascontiguousarray(dat[m]),
                "lspatch": np.ascontiguousarray(patch[m])} for m in range(M)]
    res = run_bass_kernel_spmd(nc, in_maps, core_ids=list(range(M)),
                               trace=trace)
    out = np.empty((B, N, N), dtype=np.float32)
    for m in range(M):
        o = res.results[m]["out"]            # int16 [BL*PARTS, BELEMS]
        out[m * BL:(m + 1) * BL] = o.reshape(BL, PARTS * BELEMS) \
            .view(np.float32).reshape(BL, N, N)
    return out, res.exec_time_ns


def kernel(weights, bond_src, bond_dst, bond_type, num_nodes):
    assert int(num_nodes) == N
    out, _ = run_with_stats({
        "weights": np.asarray(weights),
        "bond_src": np.asarray(bond_src),
        "bond_dst": np.asarray(bond_dst),
        "bond_type": np.asarray(bond_type),
    })
    return out
